# revision 74
# baseline (speedup 1.0000x reference)
"""Trainium2 Bass kernel for nn_MultiHeadAttention_3796751090171 (sparse_attention).

Batch-parallel SPMD across 8 NeuronCores: q_batch/k_batch are SORTED, so the
cross-batch mask makes attention block-diagonal over batches, and there are
exactly B=8 batches for 8 cores. Core c computes batch c's queries against
batch c's keys for ALL 8 heads -- completely independent work, NO collectives.

Design (trace-driven, ~74us vs the 81.6us v1 baseline; the HAM power manager
caps sustained PE throughput at ~1.3GHz average, so wall time is dominated by
PE stream columns + the saturated ACT (exp) engine):
  - exact shapes: NQ = max batch q-count, K projection only computes the real
    key count (zeros memset once for the padded tail); PE matmul cost is
    output-columns x 1 cycle.
  - all bulk inputs host-packed partition-major ([128, KT, n]) so every DMA
    moves 2-4KB contiguous runs per partition (small-packet layouts measured
    only 50-90 GB/s/queue); loads split across the sync/gpsimd/scalar queues
    by need-time; posc streamed per-head round-robin over all 3 queues.
  - V projected directly in [k, d] layout (features stationary) -- no PE
    transposes; K/Q/V projections run contraction-major so each half-tensor
    DMA chunk is consumed as it lands (4 psum banks).
  - scores -> exp (ACT) -> *exp(pos) (DVE, host-precomputed exp(pos), 2x
    mode) -> AV accumulate [hT | Z] via a fused ones-column.
  - per head: 1/Z via the fast-reciprocal DVE uop straight off the AV psum
    (full-tile: the uop mishandles partition-base-64 row APs), broadcast
    across 64 partitions via a DRAM bounce with a stride-0 partition read,
    one DVE multiply normalizes into a head-PAIR tile; the output projection
    then runs K=128 per pair into 4 persistent psum banks (no serial tail).
  - software-pipelined slot schedule (LOOK_B/LOOK_C/D2LAG) keeps PE fed
    while the ACT exp stream and the per-head Z chain run LOOK slots behind.

PSUM budget (8 banks): 2 (K proj / score tiles) + 2 (Q,V proj / AV accum) +
4 (running paired output-projection accumulators).
"""

import functools
import math

import numpy as np
import ml_dtypes

import concourse.bass as bass
import concourse.tile as tile
from concourse import bacc, mybir
from concourse.bass_utils import run_bass_kernel_spmd

N = 3072
QD = 512
OD = 512
H = 8
D = 64
B = 8
NCORES = 8
SCALE = math.sqrt(D)

F32 = mybir.dt.float32
BF16 = mybir.dt.bfloat16
BF16_NP = ml_dtypes.bfloat16

TRACE = False
LAST_RESULTS = None

# schedule tuning knobs
import os as _os
WARM = int(_os.environ.get("KWARM", "0"))   # warmup matmuls while DMAs land
LOOK_B = int(_os.environ.get("KLB", "1"))   # slots: score pair -> exp+mult
LOOK_C = int(_os.environ.get("KLC", "4"))   # slots: score pair -> AV pair
D2LAG = int(_os.environ.get("KD2", "2"))    # slots: D1 -> D2 (zbc+normalize)


def _bounds(q_batch, k_batch):
    qb = np.asarray(q_batch).astype(np.int64)
    kb = np.asarray(k_batch).astype(np.int64)
    qbound = np.searchsorted(qb, np.arange(B + 1))
    kbound = np.searchsorted(kb, np.arange(B + 1))
    return qbound, kbound


@functools.lru_cache(maxsize=8)
def _build(NQ, NKP, NKE, has_bq, has_bk, has_bv, has_bo):
    nc = bacc.Bacc("TRN2", target_bir_lowering=False, debug=False,
                   num_devices=NCORES)

    KT = QD // 128     # 4 feature-contraction chunks
    NKC = NKP // 128   # k chunks
    NTD = OD // 128    # output-dim tiles for Q/K projections

    # All bulk inputs are host-packed p-major ([128, KT, n]: partition-
    # contiguous 3-4KB runs) so the DMA engines move full-size packets --
    # 832B-row layouts measured only 50-90 GB/s per queue, 3-4KB gets ~200+.
    qfT_d = nc.dram_tensor("qfT", [128, KT, NQ], BF16, kind="ExternalInput")
    kfT_d = nc.dram_tensor("kfT", [128, KT, NKE], BF16, kind="ExternalInput")
    vfT_d = nc.dram_tensor("vfT", [128, KT, NKP], BF16, kind="ExternalInput")
    posc_d = nc.dram_tensor("posc", [H, 128, NKC, NQ], BF16, kind="ExternalInput")
    wq_d = nc.dram_tensor("wq", [128, KT, OD], BF16, kind="ExternalInput")
    wk_d = nc.dram_tensor("wk", [128, KT, OD], BF16, kind="ExternalInput")
    wv_d = nc.dram_tensor("wv", [128, KT, OD], BF16, kind="ExternalInput")
    woh_d = nc.dram_tensor("woh", [128, KT, OD], BF16, kind="ExternalInput")
    bq_d = nc.dram_tensor("bq", [128, NTD], F32, kind="ExternalInput") if has_bq else None
    bk_d = nc.dram_tensor("bk", [128, NTD], F32, kind="ExternalInput") if has_bk else None
    bv_d = nc.dram_tensor("bv", [1, OD], BF16, kind="ExternalInput") if has_bv else None
    bo_d = nc.dram_tensor("bo", [128, NTD], F32, kind="ExternalInput") if has_bo else None
    out_d = nc.dram_tensor("out", [128, NTD, NQ], BF16, kind="ExternalOutput")
    import os
    DEBUG = bool(os.environ.get("KDBG"))
    if DEBUG:
        dbg_q = nc.dram_tensor("dbg_q", [128, NTD, NQ], BF16, kind="ExternalOutput")
        dbg_k = nc.dram_tensor("dbg_k", [128, NTD, NKP], BF16, kind="ExternalOutput")
        dbg_v = nc.dram_tensor("dbg_v", [128, NKC, H, D + 1], BF16, kind="ExternalOutput")
        dbg_ht = nc.dram_tensor("dbg_ht", [H, D + 1, NQ], F32, kind="ExternalOutput")
        dbg_em = nc.dram_tensor("dbg_em", [4, 128, 2 * NQ], BF16, kind="ExternalOutput")
        dbg_hn = nc.dram_tensor("dbg_hn", [H, D, NQ], BF16, kind="ExternalOutput")
        dbg_zr = nc.dram_tensor("dbg_zr", [H, NQ], F32, kind="ExternalOutput")
        dbg_zrb = nc.dram_tensor("dbg_zrb", [H, NQ], BF16, kind="ExternalOutput")

    with tile.TileContext(nc) as tc:
        with (
            tc.tile_pool(name="consts", bufs=1) as consts,
            tc.tile_pool(name="posp", bufs=5) as posp,
            tc.tile_pool(name="expp", bufs=6) as expp,
            tc.tile_pool(name="hp", bufs=3) as hp,
            tc.tile_pool(name="hnp", bufs=2) as hnp,
            tc.tile_pool(name="outp", bufs=4) as outp,
            tc.tile_pool(name="ps_s", bufs=2, space="PSUM") as ps_s,
            tc.tile_pool(name="ps_av", bufs=2, space="PSUM") as ps_av,
            tc.tile_pool(name="ps_o", bufs=1, space="PSUM") as ps_o,
            tc.tile_pool(name="dram", bufs=1, space="DRAM") as dramp,
        ):
            # ---------------- warmup: keep the PE busy from t=0 so the HAM
            # activity ramp (1.2 -> 2.4 GHz) starts before the projections.
            # row 64 of a [65, D] ones tile: partition base matches the Z row
            # of the AV accumulator for the K=1 broadcast matmul
            if WARM:
                wtile = consts.tile([128, 128], BF16, name="wtile")
                nc.vector.memset(wtile, 0.5)
                warm_ps = ps_s.tile([128, 512], F32, tag="pss", name="warm_ps")
                for wi in range(WARM):
                    nc.tensor.matmul(warm_ps[:, 0:128], wtile[:, :],
                                     wtile[:, :], start=(wi % 8 == 0),
                                     stop=(wi % 8 == 7 or wi == WARM - 1))
                warm_sb = consts.tile([1, 1], F32, name="warm_sb")
                nc.vector.tensor_copy(warm_sb[0:1, 0:1], warm_ps[0:1, 0:1])
                warm_d = dramp.tile([1, 1], F32, name="warm_d")
                nc.gpsimd.dma_start(out=warm_d[:, :], in_=warm_sb[0:1, 0:1])

            # ---------------- weights / features (DMA issue order matters:
            # K-proj inputs first, finely chunked, so the PE starts early)
            # two half-tensor tiles per input: 2KB p-major runs keep DMA
            # packets near full rate while halving the wait for the first
            # projection matmuls; queue order follows need-time
            def half_tiles(nm, n):
                return [consts.tile([128, 2, n], BF16, tag=f"{nm}{i}",
                                    name=f"{nm}{i}") for i in range(2)]

            wk_sb = half_tiles("wk", OD)
            kf_sb = half_tiles("kf", NKE)
            wq_sb = half_tiles("wq", OD)
            qf_sb = half_tiles("qf", NQ)
            wv_sb = half_tiles("wv", OD)
            vf_sb = half_tiles("vf", NKP)
            for eng, loads in (
                (nc.sync, [(wk_sb[0], wk_d, 0), (wk_sb[1], wk_d, 1),
                           (qf_sb[0], qfT_d, 0), (qf_sb[1], qfT_d, 1)]),
                (nc.scalar, [(kf_sb[0], kfT_d, 0), (kf_sb[1], kfT_d, 1),
                             (wq_sb[0], wq_d, 0), (wq_sb[1], wq_d, 1)]),
                (nc.gpsimd, [(wv_sb[0], wv_d, 0), (wv_sb[1], wv_d, 1),
                             (vf_sb[0], vfT_d, 0), (vf_sb[1], vfT_d, 1)]),
            ):
                for dst, srcd, i in loads:
                    eng.dma_start(out=dst, in_=srcd[:, 2 * i:2 * i + 2, :])

            # Wo in natural [in, out] layout: rows 128*hp..128*hp+127 hold the
            # head-PAIR hp's input dims, enabling K=128 paired oproj matmuls
            wo_sb = consts.tile([128, NTD, OD], BF16, name="wo_sb")
            nc.gpsimd.dma_start(out=wo_sb, in_=woh_d[:, :, :])

            bias_sb = {}
            for nm, dd in (("bq", bq_d), ("bk", bk_d), ("bo", bo_d)):
                if dd is not None:
                    t = consts.tile([128, NTD], F32, tag=f"b_{nm}", name=f"b_{nm}")
                    nc.gpsimd.dma_start(out=t, in_=dd[:, :])
                    bias_sb[nm] = t
            if bv_d is not None:
                bv_sb = consts.tile([1, OD], BF16, name="bv_sb")
                nc.gpsimd.dma_start(out=bv_sb, in_=bv_d[:, :])
                ones1 = consts.tile([1, 128], BF16, name="ones1")
                nc.vector.memset(ones1, 1.0)

            # projected tensors
            KT_f = consts.tile([128, NTD, NKP], BF16, name="KT_f")
            QT_f = consts.tile([128, NTD, NQ], BF16, name="QT_f")
            V_sb = consts.tile([128, NKC, H, D + 1], BF16, name="V_sb")
            # ones column for the fused-Z row of the AV matmul
            nc.vector.memset(V_sb[:, :, :, D], 1.0)

            # ---------------- K / Q projections (out = W^T X, d on partitions)
            def proj_tmajor(f_sb, w_sb, dst, ncols, bias):
                # contraction-major: each DMA chunk t is consumed as soon as
                # it lands; needs all 4 psum banks (2 from each pool)
                tiles = [ps_s.tile([128, 512], F32, tag="pss", name="pp0"),
                         ps_s.tile([128, 512], F32, tag="pss", name="pp1"),
                         ps_av.tile([128, 512], F32, tag="avt", name="pp2"),
                         ps_av.tile([128, 512], F32, tag="avt", name="pp3")]
                for t in range(KT):
                    for td in range(NTD):
                        dsl = slice(128 * td, 128 * (td + 1))
                        nc.tensor.matmul(tiles[td][:, 0:ncols],
                                         w_sb[t // 2][:, t % 2, dsl],
                                         f_sb[t // 2][:, t % 2, 0:ncols],
                                         start=(t == 0), stop=(t == KT - 1))
                for td in range(NTD):
                    ps = tiles[td]
                    if bias is not None:
                        nc.scalar.activation(dst[:, td, 0:ncols], ps[:, 0:ncols],
                                             mybir.ActivationFunctionType.Identity,
                                             bias=bias[:, td:td + 1])
                    elif td % 2 == 0:
                        nc.scalar.copy(dst[:, td, 0:ncols], ps[:, 0:ncols])
                    else:
                        nc.vector.tensor_copy(dst[:, td, 0:ncols], ps[:, 0:ncols])

            if NKE < NKP:
                # zero the k-positions beyond the real key count once; the
                # K projection then only computes NKE columns
                nc.gpsimd.memset(KT_f[:, :, NKE:NKP], 0.0)
            proj_tmajor(kf_sb, wk_sb, KT_f, NKE, bias_sb.get("bk"))
            proj_tmajor(qf_sb, wq_sb, QT_f, NQ, bias_sb.get("bq"))

            # ---------------- V projection, direct [k, d] layout, t-major:
            # out[k, d] += vf_chunk^T @ wv_chunk  (features stationary)
            vtiles = [ps_s.tile([128, 512], F32, tag="pss", name="vp0"),
                      ps_s.tile([128, 512], F32, tag="pss", name="vp1"),
                      ps_av.tile([128, 512], F32, tag="avt", name="vp2"),
                      ps_av.tile([128, 512], F32, tag="avt", name="vp3")]
            for t in range(KT):
                for c in range(NKC):
                    ksl = slice(128 * c, 128 * (c + 1))
                    nc.tensor.matmul(vtiles[c][:, 0:OD],
                                     vf_sb[t // 2][:, t % 2, ksl],
                                     wv_sb[t // 2][:, t % 2, :],
                                     start=(t == 0),
                                     stop=(t == KT - 1 and bv_d is None))
            for c in range(NKC):
                if bv_d is not None:
                    nc.tensor.matmul(vtiles[c][:, 0:OD], ones1[:, 0:128],
                                     bv_sb[:, :], start=False, stop=True)
                src = vtiles[c][:, 0:OD].rearrange("p (h d) -> p h d", h=H)
                if c % 2 == 0:
                    nc.scalar.copy(V_sb[:, c, :, 0:D], src)
                else:
                    nc.vector.tensor_copy(V_sb[:, c, :, 0:D], src)

            # ---------------- attention: slots = (head, kchunk-pair) --------
            NP = NKC // 2          # kchunk pairs per head (2)
            S = H * NP             # 16 slots
            st_ps = {}
            expm = {}
            avps = {}
            hts = {}
            pos_tiles = {}
            pos_eng = [nc.gpsimd, nc.sync, nc.gpsimd]

            def issue_pos(h):
                t = posp.tile([128, NKC, NQ], BF16, tag="pos", name="pos")
                pos_eng[h % 3].dma_start(out=t, in_=posc_d[h])
                pos_tiles[h] = t

            for h in range(min(4, H)):
                issue_pos(h)

            def stageA(s):  # score pair matmuls (two f32 psum tiles)
                h, p = s // NP, s % NP
                if p == 0 and h + 4 < H:
                    issue_pos(h + 4)
                po = D * (h % 2)
                tiles = []
                for ci in (0, 1):
                    c = 2 * p + ci
                    ksl = slice(128 * c, 128 * (c + 1))
                    pst = ps_s.tile([128, NQ], F32, tag="pss", name="pst")
                    nc.tensor.matmul(pst[:, 0:NQ],
                                     KT_f[po:po + D, h // 2, ksl],
                                     QT_f[po:po + D, h // 2, 0:NQ],
                                     start=True, stop=True)
                    tiles.append(pst)
                st_ps[s] = tiles

            def stageD1(h):  # 1/Z off the AV psum; bounce it via DRAM to
                # broadcast across 64 partitions (stride-0 partition read) --
                # no PE matmul, no ACT convert.
                # full-tile recip: the custom DVE uop mishandles partition-
                # base-64 single-row APs; lanes are parallel so [65,NQ] costs
                # the same and row 64 gives 1/Z (rows 0..63 are junk, unread)
                zr = hp.tile([D + 1, NQ], F32, tag="zr", name="zr")
                nc.vector.reciprocal_approx_fast(zr[:, :], avps[h][:, 0:NQ])
                zr_d = dramp.tile([1, NQ], F32, tag=f"zrd{h}", name=f"zrd{h}")
                nc.gpsimd.dma_start(out=zr_d[0:1, :], in_=zr[D:D + 1, :])
                zr_ap = zr_d[:, :]
                zbc = hp.tile([D, NQ], F32, tag="zbc", name="zbc")
                nc.sync.dma_start(
                    out=zbc[:, :],
                    in_=bass.AP(tensor=zr_ap.tensor, offset=zr_ap.offset,
                                ap=[[0, D], [1, NQ]]))
                ht = hp.tile([D + 1, NQ], F32, tag="hT", name="ht")
                if h % 4 == 0:
                    nc.scalar.copy(ht[:, :], avps[h][:, 0:NQ])
                else:
                    nc.vector.tensor_copy(ht[:, :], avps[h][:, 0:NQ])
                del avps[h]
                if DEBUG:
                    nc.sync.dma_start(out=dbg_ht.ap()[h], in_=ht[:, :])
                    nc.sync.dma_start(out=dbg_zr.ap()[h:h + 1, :],
                                      in_=zr[D:D + 1, :])
                hts[h] = (ht, zbc)

            def stageB(s):  # exp x2 (ACT) + paired posc multiply (DVE)
                h, p = s // NP, s % NP
                ex = expp.tile([128, 2 * NQ], BF16, tag="expr", name="ex")
                for ci in (0, 1):
                    nc.scalar.activation(ex[:, NQ * ci:NQ * (ci + 1)],
                                         st_ps[s][ci][:, 0:NQ],
                                         mybir.ActivationFunctionType.Exp)
                del st_ps[s]
                em = expp.tile([128, 2 * NQ], BF16, tag="expm", name="em")
                nc.vector.tensor_mul(
                    em[:, :].rearrange("p (c q) -> p c q", c=2),
                    ex[:, :].rearrange("p (c q) -> p c q", c=2),
                    pos_tiles[h][:, 2 * p:2 * p + 2, :])
                if DEBUG and s < 4:
                    nc.sync.dma_start(out=dbg_em.ap()[s], in_=em[:, :])
                expm[s] = em

            def stageC(s):  # AV pair (accumulate [hT | Z] per head)
                h, p = s // NP, s % NP
                if p == 0:
                    avps[h] = ps_av.tile([D + 1, NQ], F32, tag="avt", name="avt")
                for ci in (0, 1):
                    c = 2 * p + ci
                    nc.tensor.matmul(avps[h][:, 0:NQ], V_sb[:, c, h, :],
                                     expm[s][:, NQ * ci:NQ * (ci + 1)],
                                     start=(c == 0), stop=(c == NKC - 1))
                del expm[s]

            hn2 = {}

            def stageD2(h):  # normalize into the pair tile
                ht, zbc = hts.pop(h)
                hp_ = h // 2
                if h % 2 == 0:
                    hn2[hp_] = hnp.tile([128, NQ], BF16, tag="hTn", name="hn")
                po = D * (h % 2)
                nc.vector.tensor_mul(hn2[hp_][po:po + D, :], ht[0:D, :],
                                     zbc[:, :])
                if DEBUG:
                    nc.sync.dma_start(out=dbg_hn.ap()[h],
                                      in_=hn2[hp_][po:po + D, :])

            def stageD3(hp_):  # paired output projection (K=128)
                hn = hn2.pop(hp_)
                last = hp_ == H // 2 - 1
                for oc in range(NTD):
                    nc.tensor.matmul(ps_o_t[oc][:, 0:NQ],
                                     wo_sb[:, hp_, 128 * oc:128 * (oc + 1)],
                                     hn[:, :],
                                     start=(hp_ == 0), stop=(hp_ == H // 2 - 1),
                                     skip_group_check=True)
                    if last:
                        store_out(oc)

            ps_o_t = [ps_o.tile([128, NQ], F32, tag=f"o{oc}", name=f"po{oc}")
                      for oc in range(NTD)]

            def store_out(oc):
                osb = outp.tile([128, NQ], BF16, tag="osb", name="osb")
                if bo_d is not None:
                    nc.scalar.activation(osb[:, :], ps_o_t[oc][:, 0:NQ],
                                         mybir.ActivationFunctionType.Identity,
                                         bias=bias_sb["bo"][:, oc:oc + 1])
                elif oc % 2 == 0:
                    nc.scalar.copy(osb[:, :], ps_o_t[oc][:, 0:NQ])
                else:
                    nc.vector.tensor_copy(osb[:, :], ps_o_t[oc][:, 0:NQ])
                (nc.sync if oc % 2 == 0 else nc.gpsimd).dma_start(
                    out=out_d[:, oc, 0:NQ], in_=osb[:, :])

            # D1(h) fires in the same slot as its p1 AV pair (right after it);
            # D2(h) D2LAG slots later (PE visits the zbc matmul after slots of
            # score/AV work, hiding the recip->zrb latency); D3 (paired oproj)
            # one slot after the odd head's D2.
            d2base = 2 * NP + 1 + LOOK_C + D2LAG  # slot of D2(h=NP-...)? h=1
            # D1(h) at 2h+1+LOOK_C; D2(h) at 2h+1+LOOK_C+D2LAG;
            # D3(hp) at 2(2hp+1)+1+LOOK_C+D2LAG+1
            for s in range(S + LOOK_C + D2LAG + 6):
                if s < S:
                    stageA(s)
                if 0 <= s - LOOK_B < S:
                    stageB(s - LOOK_B)
                sd = s - LOOK_C  # slot whose AV-pair is issued now
                if 0 <= sd < S:
                    stageC(sd)
                    if sd % NP == NP - 1:
                        stageD1(sd // NP)
                o2 = s - (1 + LOOK_C + D2LAG)
                if o2 >= 0 and o2 % 2 == 0 and o2 // 2 < H:
                    stageD2(o2 // 2)
                o3 = s - (3 + LOOK_C + D2LAG + 1)
                if o3 >= 0 and o3 % 4 == 0 and o3 // 4 < H // 2:
                    stageD3(o3 // 4)

            if DEBUG:
                nc.sync.dma_start(out=dbg_q.ap(), in_=QT_f[:, :, :])
                nc.sync.dma_start(out=dbg_k.ap(), in_=KT_f[:, :, :])
                nc.sync.dma_start(out=dbg_v.ap(), in_=V_sb[:, :, :, :])

    nc.compile()
    return nc


def _kernel_numpy(q_feat, k_feat, v_feat, pos_enc, Wq, bq, Wk, bk, Wv, bv,
                  Wo, bo, q_batch, k_batch):
    """Host fallback (degenerate batch layouts)."""
    Q = (q_feat @ Wq + bq).reshape(N, H, D).transpose(1, 0, 2)
    K = (k_feat @ Wk + bk).reshape(N, H, D).transpose(1, 0, 2)
    V = (v_feat @ Wv + bv).reshape(N, H, D).transpose(1, 0, 2)
    scores = np.einsum("hnd,hmd->hnm", Q, K) / SCALE + pos_enc
    mask = q_batch[:, None] != k_batch[None, :]
    scores = np.where(mask[None], np.float32(-1e9), scores)
    scores = scores - scores.max(-1, keepdims=True)
    e = np.exp(scores)
    probs = e / e.sum(-1, keepdims=True)
    h = np.einsum("hnm,hmd->hnd", probs, V)
    h = h.transpose(1, 0, 2).reshape(N, OD)
    return (h @ Wo + bo).astype(np.float32)


def kernel(q_feat, k_feat, v_feat, pos_enc, Wq, bq, Wk, bk, Wv, bv, Wo, bo,
           q_batch, k_batch):
    global LAST_RESULTS
    args = dict(q_feat=np.asarray(q_feat, np.float32),
                k_feat=np.asarray(k_feat, np.float32),
                v_feat=np.asarray(v_feat, np.float32),
                pos_enc=np.asarray(pos_enc, np.float32),
                Wq=np.asarray(Wq, np.float32), bq=np.asarray(bq, np.float32),
                Wk=np.asarray(Wk, np.float32), bk=np.asarray(bk, np.float32),
                Wv=np.asarray(Wv, np.float32), bv=np.asarray(bv, np.float32),
                Wo=np.asarray(Wo, np.float32), bo=np.asarray(bo, np.float32),
                q_batch=np.asarray(q_batch), k_batch=np.asarray(k_batch))

    qbound, kbound = _bounds(args["q_batch"], args["k_batch"])
    nq_all = np.diff(qbound)
    nk_all = np.diff(kbound)
    if np.any((nq_all > 0) & (nk_all == 0)) or nq_all.max() == 0:
        # a batch with queries but no keys gets uniform attention over ALL
        # keys in the reference; fall back (never happens for real inputs)
        return _kernel_numpy(**args)

    NQ = int(nq_all.max())
    NKE = int(nk_all.max())
    NKP = max(128, ((NKE + 127) // 128) * 128)
    if NQ > 512 or NKP > 512:
        return _kernel_numpy(**args)

    has_bq = bool(np.any(args["bq"]))
    has_bk = bool(np.any(args["bk"]))
    has_bv = bool(np.any(args["bv"]))
    has_bo = bool(np.any(args["bo"]))

    nc = _build(NQ, NKP, NKE, has_bq, has_bk, has_bv, has_bo)

    NKC = NKP // 128
    NTD = OD // 128

    # ---- host-side sharding / layout / padding ----
    def pmaj(x):
        # [KT*128, n] -> [128, KT, n] partition-major (3-4KB DMA runs)
        kt = x.shape[0] // 128
        return np.ascontiguousarray(
            x.reshape(kt, 128, x.shape[1]).transpose(1, 0, 2))

    qfT = np.ascontiguousarray(args["q_feat"].T).astype(BF16_NP)
    kfT = np.ascontiguousarray(args["k_feat"].T).astype(BF16_NP)
    vfT = np.ascontiguousarray(args["v_feat"].T).astype(BF16_NP)
    wq8 = pmaj((args["Wq"] / SCALE).astype(BF16_NP))
    wkb = pmaj(args["Wk"].astype(BF16_NP))
    wvb = pmaj(args["Wv"].astype(BF16_NP))
    woh = pmaj(args["Wo"].astype(BF16_NP))

    in_maps = []
    for c in range(NCORES):
        qs, qe = int(qbound[c]), int(qbound[c + 1])
        ks, ke = int(kbound[c]), int(kbound[c + 1])
        nq, nk = qe - qs, ke - ks

        qfc = np.zeros((QD, NQ), BF16_NP)
        qfc[:, :nq] = qfT[:, qs:qe]
        kfc = np.zeros((QD, NKE), BF16_NP)
        kfc[:, :nk] = kfT[:, ks:ke]
        vfc = np.zeros((QD, NKP), BF16_NP)
        vfc[:, :nk] = vfT[:, ks:ke]
        qfc, kfc, vfc = pmaj(qfc), pmaj(kfc), pmaj(vfc)

        # posc holds exp(pos): 0 on masked/pad k rows, 1 on pad-q columns
        posc = np.zeros((H, NKP, NQ), BF16_NP)
        if nk > 0:
            posc[:, :nk, :] = 1.0
            posc[:, :nk, :nq] = np.exp(
                args["pos_enc"][:, qs:qe, ks:ke]).swapaxes(1, 2).astype(BF16_NP)
        # [H, NKP, NQ] -> [H, 128, NKC, NQ] p-major per head
        nkc = NKP // 128
        posc = np.ascontiguousarray(
            posc.reshape(H, nkc, 128, NQ).transpose(0, 2, 1, 3))

        m = {"qfT": qfc, "kfT": kfc, "vfT": vfc, "posc": posc,
             "wq": wq8, "wk": wkb, "wv": wvb, "woh": woh}
        if has_bq:
            m["bq"] = np.ascontiguousarray(
                (args["bq"] / SCALE).astype(np.float32).reshape(NTD, 128).T)
        if has_bk:
            m["bk"] = np.ascontiguousarray(
                args["bk"].astype(np.float32).reshape(NTD, 128).T)
        if has_bv:
            m["bv"] = args["bv"].astype(BF16_NP).reshape(1, OD)
        if has_bo:
            m["bo"] = np.ascontiguousarray(
                args["bo"].astype(np.float32).reshape(NTD, 128).T)
        in_maps.append(m)

    res = run_bass_kernel_spmd(nc, in_maps, core_ids=list(range(NCORES)),
                               trace=TRACE)
    LAST_RESULTS = res
    out = np.empty((N, OD), np.float32)
    for c in range(NCORES):
        qs, qe = int(qbound[c]), int(qbound[c + 1])
        if qe > qs:
            # out is [128, NTD, NQ] p-major -> [OD, NQ]
            o = res.results[c]["out"].transpose(1, 0, 2).reshape(OD, NQ)
            out[qs:qe, :] = o[:, :qe - qs].T.astype(np.float32)
    return out


# revision 75
# speedup vs baseline: 1.1321x; 1.1321x over previous
"""Trainium2 Bass kernel for nn_MultiHeadAttention_3796751090171 (sparse_attention).

Batch-parallel SPMD across 8 NeuronCores: q_batch/k_batch are SORTED, so the
cross-batch mask makes attention block-diagonal over batches, and there are
exactly B=8 batches for 8 cores. Core c computes batch c's queries against
batch c's keys for ALL 8 heads -- completely independent work, NO collectives.

Design (trace-driven, ~74us vs the 81.6us v1 baseline; the HAM power manager
caps sustained PE throughput at ~1.3GHz average, so wall time is dominated by
PE stream columns + the saturated ACT (exp) engine):
  - exact shapes: NQ = max batch q-count, K projection only computes the real
    key count (zeros memset once for the padded tail); PE matmul cost is
    output-columns x 1 cycle.
  - all bulk inputs host-packed partition-major ([128, KT, n]) so every DMA
    moves 2-4KB contiguous runs per partition (small-packet layouts measured
    only 50-90 GB/s/queue); loads split across the sync/gpsimd/scalar queues
    by need-time; posc streamed per-head round-robin over all 3 queues.
  - V projected directly in [k, d] layout (features stationary) -- no PE
    transposes; K/Q/V projections run contraction-major so each half-tensor
    DMA chunk is consumed as it lands (4 psum banks).
  - scores -> exp (ACT) -> *exp(pos) (DVE, host-precomputed exp(pos), 2x
    mode) -> AV accumulate [hT | Z] via a fused ones-column.
  - per head: 1/Z via the fast-reciprocal DVE uop straight off the AV psum
    (full-tile: the uop mishandles partition-base-64 row APs), broadcast
    across 64 partitions via a DRAM bounce with a stride-0 partition read,
    one DVE multiply normalizes into a head-PAIR tile; the output projection
    then runs K=128 per pair into 4 persistent psum banks (no serial tail).
  - software-pipelined slot schedule (LOOK_B/LOOK_C/D2LAG) keeps PE fed
    while the ACT exp stream and the per-head Z chain run LOOK slots behind.

PSUM budget (8 banks): 2 (K proj / score tiles) + 2 (Q,V proj / AV accum) +
4 (running paired output-projection accumulators).
"""

import functools
import math

import numpy as np
import ml_dtypes

import concourse.bass as bass
import concourse.tile as tile
from concourse import bacc, mybir
from concourse.bass_utils import run_bass_kernel_spmd

N = 3072
QD = 512
OD = 512
H = 8
D = 64
B = 8
NCORES = 8
SCALE = math.sqrt(D)

F32 = mybir.dt.float32
BF16 = mybir.dt.bfloat16
BF16_NP = ml_dtypes.bfloat16

TRACE = False
LAST_RESULTS = None

# schedule tuning knobs
import os as _os
WARM = int(_os.environ.get("KWARM", "0"))   # warmup matmuls while DMAs land
LOOK_B = int(_os.environ.get("KLB", "1"))   # slots: score pair -> exp+mult
LOOK_C = int(_os.environ.get("KLC", "4"))   # slots: score pair -> AV pair
D2LAG = int(_os.environ.get("KD2", "2"))    # slots: D1 -> D2 (zbc+normalize)


def _bounds(q_batch, k_batch):
    qb = np.asarray(q_batch).astype(np.int64)
    kb = np.asarray(k_batch).astype(np.int64)
    qbound = np.searchsorted(qb, np.arange(B + 1))
    kbound = np.searchsorted(kb, np.arange(B + 1))
    return qbound, kbound


@functools.lru_cache(maxsize=8)
def _build(NQ, NKP, NKE, has_bq, has_bk, has_bv, has_bo):
    nc = bacc.Bacc("TRN2", target_bir_lowering=False, debug=False,
                   num_devices=NCORES)

    KT = QD // 128     # 4 feature-contraction chunks
    NKC = NKP // 128   # k chunks
    NTD = OD // 128    # output-dim tiles for Q/K projections

    # All bulk inputs are host-packed p-major ([128, KT, n]: partition-
    # contiguous 3-4KB runs) so the DMA engines move full-size packets --
    # 832B-row layouts measured only 50-90 GB/s per queue, 3-4KB gets ~200+.
    qfT_d = nc.dram_tensor("qfT", [128, KT, NQ], BF16, kind="ExternalInput")
    kfT_d = nc.dram_tensor("kfT", [128, KT, NKE], BF16, kind="ExternalInput")
    vfT_d = nc.dram_tensor("vfT", [128, KT, NKP], BF16, kind="ExternalInput")
    posc_d = nc.dram_tensor("posc", [H, 128, NKC, NQ], BF16, kind="ExternalInput")
    wq_d = nc.dram_tensor("wq", [128, KT, OD], BF16, kind="ExternalInput")
    wk_d = nc.dram_tensor("wk", [128, KT, OD], BF16, kind="ExternalInput")
    wv_d = nc.dram_tensor("wv", [128, KT, OD], BF16, kind="ExternalInput")
    woh_d = nc.dram_tensor("woh", [128, KT, OD], BF16, kind="ExternalInput")
    bq_d = nc.dram_tensor("bq", [128, NTD], F32, kind="ExternalInput") if has_bq else None
    bk_d = nc.dram_tensor("bk", [128, NTD], F32, kind="ExternalInput") if has_bk else None
    bv_d = nc.dram_tensor("bv", [1, OD], BF16, kind="ExternalInput") if has_bv else None
    bo_d = nc.dram_tensor("bo", [128, NTD], F32, kind="ExternalInput") if has_bo else None
    out_d = nc.dram_tensor("out", [128, NTD, NQ], BF16, kind="ExternalOutput")
    import os
    DEBUG = bool(os.environ.get("KDBG"))
    if DEBUG:
        dbg_q = nc.dram_tensor("dbg_q", [128, NTD, NQ], BF16, kind="ExternalOutput")
        dbg_k = nc.dram_tensor("dbg_k", [128, NTD, NKP], BF16, kind="ExternalOutput")
        dbg_v = nc.dram_tensor("dbg_v", [128, NKC, H, D + 1], BF16, kind="ExternalOutput")
        dbg_ht = nc.dram_tensor("dbg_ht", [H, D + 1, NQ], F32, kind="ExternalOutput")
        dbg_em = nc.dram_tensor("dbg_em", [4, 128, 2 * NQ], BF16, kind="ExternalOutput")
        dbg_hn = nc.dram_tensor("dbg_hn", [H, D, NQ], BF16, kind="ExternalOutput")
        dbg_zr = nc.dram_tensor("dbg_zr", [H, NQ], F32, kind="ExternalOutput")
        dbg_zrb = nc.dram_tensor("dbg_zrb", [H, NQ], BF16, kind="ExternalOutput")

    with tile.TileContext(nc) as tc:
        with (
            tc.tile_pool(name="consts", bufs=1) as consts,
            tc.tile_pool(name="posp", bufs=4) as posp,
            tc.tile_pool(name="expp", bufs=4) as expp,
            tc.tile_pool(name="hp", bufs=3) as hp,
            tc.tile_pool(name="hnp", bufs=2) as hnp,
            tc.tile_pool(name="outp", bufs=4) as outp,
            tc.tile_pool(name="ps_s", bufs=2, space="PSUM") as ps_s,
            tc.tile_pool(name="ps_av", bufs=2, space="PSUM") as ps_av,
            tc.tile_pool(name="ps_o", bufs=1, space="PSUM") as ps_o,
            tc.tile_pool(name="dram", bufs=1, space="DRAM") as dramp,
        ):
            # ---------------- warmup: keep the PE busy from t=0 so the HAM
            # activity ramp (1.2 -> 2.4 GHz) starts before the projections.
            # row 64 of a [65, D] ones tile: partition base matches the Z row
            # of the AV accumulator for the K=1 broadcast matmul
            if WARM:
                wtile = consts.tile([128, 128], BF16, name="wtile")
                nc.vector.memset(wtile, 0.5)
                warm_ps = ps_s.tile([128, 512], F32, tag="pss", name="warm_ps")
                for wi in range(WARM):
                    nc.tensor.matmul(warm_ps[:, 0:128], wtile[:, :],
                                     wtile[:, :], start=(wi % 8 == 0),
                                     stop=(wi % 8 == 7 or wi == WARM - 1))
                warm_sb = consts.tile([1, 1], F32, name="warm_sb")
                nc.vector.tensor_copy(warm_sb[0:1, 0:1], warm_ps[0:1, 0:1])
                warm_d = dramp.tile([1, 1], F32, name="warm_d")
                nc.gpsimd.dma_start(out=warm_d[:, :], in_=warm_sb[0:1, 0:1])

            # ---------------- weights / features (DMA issue order matters:
            # K-proj inputs first, finely chunked, so the PE starts early)
            # two half-tensor tiles per input: 2KB p-major runs keep DMA
            # packets near full rate while halving the wait for the first
            # projection matmuls; queue order follows need-time
            def half_tiles(nm, n):
                return [consts.tile([128, 2, n], BF16, tag=f"{nm}{i}",
                                    name=f"{nm}{i}") for i in range(2)]

            wk_sb = half_tiles("wk", OD)
            kf_sb = half_tiles("kf", NKE)
            wq_sb = half_tiles("wq", OD)
            qf_sb = half_tiles("qf", NQ)
            wv_sb = half_tiles("wv", OD)
            vf_sb = half_tiles("vf", NKP)
            for eng, loads in (
                (nc.sync, [(wk_sb[0], wk_d, 0), (wk_sb[1], wk_d, 1),
                           (qf_sb[0], qfT_d, 0), (qf_sb[1], qfT_d, 1)]),
                (nc.scalar, [(kf_sb[0], kfT_d, 0), (kf_sb[1], kfT_d, 1),
                             (wq_sb[0], wq_d, 0), (wq_sb[1], wq_d, 1)]),
                (nc.gpsimd, [(wv_sb[0], wv_d, 0), (wv_sb[1], wv_d, 1),
                             (vf_sb[0], vfT_d, 0), (vf_sb[1], vfT_d, 1)]),
            ):
                for dst, srcd, i in loads:
                    eng.dma_start(out=dst, in_=srcd[:, 2 * i:2 * i + 2, :])

            # Wo in natural [in, out] layout: rows 128*hp..128*hp+127 hold the
            # head-PAIR hp's input dims, enabling K=128 paired oproj matmuls
            wo_sb = consts.tile([128, NTD, OD], BF16, name="wo_sb")
            nc.gpsimd.dma_start(out=wo_sb, in_=woh_d[:, :, :])

            bias_sb = {}
            for nm, dd in (("bq", bq_d), ("bk", bk_d), ("bo", bo_d)):
                if dd is not None:
                    t = consts.tile([128, NTD], F32, tag=f"b_{nm}", name=f"b_{nm}")
                    nc.gpsimd.dma_start(out=t, in_=dd[:, :])
                    bias_sb[nm] = t
            if bv_d is not None:
                bv_sb = consts.tile([1, OD], BF16, name="bv_sb")
                nc.gpsimd.dma_start(out=bv_sb, in_=bv_d[:, :])
                ones1 = consts.tile([1, 128], BF16, name="ones1")
                nc.vector.memset(ones1, 1.0)

            # projected tensors
            KT_f = consts.tile([128, NTD, NKP], BF16, name="KT_f")
            QT_f = consts.tile([128, NTD, NQ], BF16, name="QT_f")
            V_sb = consts.tile([128, NKC, H, D + 1], BF16, name="V_sb")
            # ones column for the fused-Z row of the AV matmul
            nc.vector.memset(V_sb[:, :, :, D], 1.0)

            # ---------------- K / Q projections (out = W^T X, d on partitions)
            def proj_tmajor(f_sb, w_sb, dst, ncols, bias):
                # contraction-major: each DMA chunk t is consumed as soon as
                # it lands; needs all 4 psum banks (2 from each pool)
                tiles = [ps_s.tile([128, 512], F32, tag="pss", name="pp0"),
                         ps_s.tile([128, 512], F32, tag="pss", name="pp1"),
                         ps_av.tile([128, 512], F32, tag="avt", name="pp2"),
                         ps_av.tile([128, 512], F32, tag="avt", name="pp3")]
                for t in range(KT):
                    for td in range(NTD):
                        dsl = slice(128 * td, 128 * (td + 1))
                        nc.tensor.matmul(tiles[td][:, 0:ncols],
                                         w_sb[t // 2][:, t % 2, dsl],
                                         f_sb[t // 2][:, t % 2, 0:ncols],
                                         start=(t == 0), stop=(t == KT - 1))
                for td in range(NTD):
                    ps = tiles[td]
                    if bias is not None:
                        nc.scalar.activation(dst[:, td, 0:ncols], ps[:, 0:ncols],
                                             mybir.ActivationFunctionType.Identity,
                                             bias=bias[:, td:td + 1])
                    elif td % 2 == 0:
                        nc.scalar.copy(dst[:, td, 0:ncols], ps[:, 0:ncols])
                    else:
                        nc.vector.tensor_copy(dst[:, td, 0:ncols], ps[:, 0:ncols])

            if NKE < NKP:
                # zero the k-positions beyond the real key count once; the
                # K projection then only computes NKE columns
                nc.gpsimd.memset(KT_f[:, :, NKE:NKP], 0.0)
            proj_tmajor(kf_sb, wk_sb, KT_f, NKE, bias_sb.get("bk"))
            proj_tmajor(qf_sb, wq_sb, QT_f, NQ, bias_sb.get("bq"))

            # ---------------- V projection, direct [k, d] layout, t-major:
            # out[k, d] += vf_chunk^T @ wv_chunk  (features stationary)
            vtiles = [ps_s.tile([128, 512], F32, tag="pss", name="vp0"),
                      ps_s.tile([128, 512], F32, tag="pss", name="vp1"),
                      ps_av.tile([128, 512], F32, tag="avt", name="vp2"),
                      ps_av.tile([128, 512], F32, tag="avt", name="vp3")]
            for t in range(KT):
                for c in range(NKC):
                    ksl = slice(128 * c, 128 * (c + 1))
                    nc.tensor.matmul(vtiles[c][:, 0:OD],
                                     vf_sb[t // 2][:, t % 2, ksl],
                                     wv_sb[t // 2][:, t % 2, :],
                                     start=(t == 0),
                                     stop=(t == KT - 1 and bv_d is None))
            for c in range(NKC):
                if bv_d is not None:
                    nc.tensor.matmul(vtiles[c][:, 0:OD], ones1[:, 0:128],
                                     bv_sb[:, :], start=False, stop=True)
                src = vtiles[c][:, 0:OD].rearrange("p (h d) -> p h d", h=H)
                if c % 2 == 0:
                    nc.scalar.copy(V_sb[:, c, :, 0:D], src)
                else:
                    nc.vector.tensor_copy(V_sb[:, c, :, 0:D], src)

            # ---------------- attention: slots = (head, kchunk-pair) --------
            NP = NKC // 2          # kchunk pairs per head (2)
            S = H * NP             # 16 slots
            st_ps = {}
            expm = {}
            avps = {}
            hts = {}
            pos_tiles = {}
            pos_eng = [nc.gpsimd, nc.sync, nc.gpsimd]

            def issue_pos(h):
                t = posp.tile([128, NKC, NQ], BF16, tag="pos", name="pos")
                pos_eng[h % 3].dma_start(out=t, in_=posc_d[h])
                pos_tiles[h] = t

            for h in range(min(3, H)):
                issue_pos(h)

            def stageA(s):  # score pair matmuls (two f32 psum tiles)
                h, p = s // NP, s % NP
                if p == 0 and h + 3 < H:
                    issue_pos(h + 3)
                po = D * (h % 2)
                tiles = []
                for ci in (0, 1):
                    c = 2 * p + ci
                    ksl = slice(128 * c, 128 * (c + 1))
                    pst = ps_s.tile([128, NQ], F32, tag="pss", name="pst")
                    nc.tensor.matmul(pst[:, 0:NQ],
                                     KT_f[po:po + D, h // 2, ksl],
                                     QT_f[po:po + D, h // 2, 0:NQ],
                                     start=True, stop=True)
                    tiles.append(pst)
                st_ps[s] = tiles

            def stageD1(h):  # 1/Z off the AV psum; bounce it via DRAM to
                # broadcast across 64 partitions (stride-0 partition read) --
                # no PE matmul, no ACT convert.
                # full-tile recip: the custom DVE uop mishandles partition-
                # base-64 single-row APs; lanes are parallel so [65,NQ] costs
                # the same and row 64 gives 1/Z (rows 0..63 are junk, unread)
                zr = hp.tile([D + 1, NQ], F32, tag="zr", name="zr")
                nc.vector.reciprocal_approx_fast(zr[:, :], avps[h][:, 0:NQ])
                zr_d = dramp.tile([1, NQ], F32, tag=f"zrd{h}", name=f"zrd{h}")
                nc.gpsimd.dma_start(out=zr_d[0:1, :], in_=zr[D:D + 1, :])
                zr_ap = zr_d[:, :]
                zbc = hp.tile([D, NQ], F32, tag="zbc", name="zbc")
                nc.sync.dma_start(
                    out=zbc[:, :],
                    in_=bass.AP(tensor=zr_ap.tensor, offset=zr_ap.offset,
                                ap=[[0, D], [1, NQ]]))
                ht = hp.tile([D + 1, NQ], F32, tag="hT", name="ht")
                if h % 4 == 0:
                    nc.scalar.copy(ht[:, :], avps[h][:, 0:NQ])
                else:
                    nc.vector.tensor_copy(ht[:, :], avps[h][:, 0:NQ])
                del avps[h]
                if DEBUG:
                    nc.sync.dma_start(out=dbg_ht.ap()[h], in_=ht[:, :])
                    nc.sync.dma_start(out=dbg_zr.ap()[h:h + 1, :],
                                      in_=zr[D:D + 1, :])
                hts[h] = (ht, zbc)

            def stageB(s):  # exp x2 (ACT) + paired posc multiply (DVE)
                h, p = s // NP, s % NP
                ex = expp.tile([128, 2 * NQ], BF16, tag="expr", name="ex")
                for ci in (0, 1):
                    nc.scalar.activation(ex[:, NQ * ci:NQ * (ci + 1)],
                                         st_ps[s][ci][:, 0:NQ],
                                         mybir.ActivationFunctionType.Exp)
                del st_ps[s]
                em = expp.tile([128, 2 * NQ], BF16, tag="expm", name="em")
                nc.vector.tensor_mul(
                    em[:, :].rearrange("p (c q) -> p c q", c=2),
                    ex[:, :].rearrange("p (c q) -> p c q", c=2),
                    pos_tiles[h][:, 2 * p:2 * p + 2, :])
                if DEBUG and s < 4:
                    nc.sync.dma_start(out=dbg_em.ap()[s], in_=em[:, :])
                expm[s] = em

            def stageC(s):  # AV pair (accumulate [hT | Z] per head)
                h, p = s // NP, s % NP
                if p == 0:
                    avps[h] = ps_av.tile([D + 1, NQ], F32, tag="avt", name="avt")
                for ci in (0, 1):
                    c = 2 * p + ci
                    nc.tensor.matmul(avps[h][:, 0:NQ], V_sb[:, c, h, :],
                                     expm[s][:, NQ * ci:NQ * (ci + 1)],
                                     start=(c == 0), stop=(c == NKC - 1))
                del expm[s]

            hn2 = {}

            def stageD2(h):  # normalize into the pair tile
                ht, zbc = hts.pop(h)
                hp_ = h // 2
                if h % 2 == 0:
                    hn2[hp_] = hnp.tile([128, NQ], BF16, tag="hTn", name="hn")
                po = D * (h % 2)
                nc.vector.tensor_mul(hn2[hp_][po:po + D, :], ht[0:D, :],
                                     zbc[:, :])
                if DEBUG:
                    nc.sync.dma_start(out=dbg_hn.ap()[h],
                                      in_=hn2[hp_][po:po + D, :])

            def stageD3(hp_):  # paired output projection (K=128)
                hn = hn2.pop(hp_)
                last = hp_ == H // 2 - 1
                for oc in range(NTD):
                    nc.tensor.matmul(ps_o_t[oc][:, 0:NQ],
                                     wo_sb[:, hp_, 128 * oc:128 * (oc + 1)],
                                     hn[:, :],
                                     start=(hp_ == 0), stop=(hp_ == H // 2 - 1),
                                     skip_group_check=True)
                    if last:
                        store_out(oc)

            ps_o_t = [ps_o.tile([128, NQ], F32, tag=f"o{oc}", name=f"po{oc}")
                      for oc in range(NTD)]

            def store_out(oc):
                osb = outp.tile([128, NQ], BF16, tag="osb", name="osb")
                if bo_d is not None:
                    nc.scalar.activation(osb[:, :], ps_o_t[oc][:, 0:NQ],
                                         mybir.ActivationFunctionType.Identity,
                                         bias=bias_sb["bo"][:, oc:oc + 1])
                elif oc % 2 == 0:
                    nc.scalar.copy(osb[:, :], ps_o_t[oc][:, 0:NQ])
                else:
                    nc.vector.tensor_copy(osb[:, :], ps_o_t[oc][:, 0:NQ])
                (nc.sync if oc % 2 == 0 else nc.gpsimd).dma_start(
                    out=out_d[:, oc, 0:NQ], in_=osb[:, :])

            # D1(h) fires in the same slot as its p1 AV pair (right after it);
            # D2(h) D2LAG slots later (PE visits the zbc matmul after slots of
            # score/AV work, hiding the recip->zrb latency); D3 (paired oproj)
            # one slot after the odd head's D2.
            d2base = 2 * NP + 1 + LOOK_C + D2LAG  # slot of D2(h=NP-...)? h=1
            # D1(h) at 2h+1+LOOK_C; D2(h) at 2h+1+LOOK_C+D2LAG;
            # D3(hp) at 2(2hp+1)+1+LOOK_C+D2LAG+1
            for s in range(S + LOOK_C + D2LAG + 6):
                if s < S:
                    stageA(s)
                if 0 <= s - LOOK_B < S:
                    stageB(s - LOOK_B)
                sd = s - LOOK_C  # slot whose AV-pair is issued now
                if 0 <= sd < S:
                    stageC(sd)
                    if sd % NP == NP - 1:
                        stageD1(sd // NP)
                o2 = s - (1 + LOOK_C + D2LAG)
                if o2 >= 0 and o2 % 2 == 0 and o2 // 2 < H:
                    stageD2(o2 // 2)
                o3 = s - (3 + LOOK_C + D2LAG + 1)
                if o3 >= 0 and o3 % 4 == 0 and o3 // 4 < H // 2:
                    stageD3(o3 // 4)

            if DEBUG:
                nc.sync.dma_start(out=dbg_q.ap(), in_=QT_f[:, :, :])
                nc.sync.dma_start(out=dbg_k.ap(), in_=KT_f[:, :, :])
                nc.sync.dma_start(out=dbg_v.ap(), in_=V_sb[:, :, :, :])

    nc.compile()
    return nc


def _kernel_numpy(q_feat, k_feat, v_feat, pos_enc, Wq, bq, Wk, bk, Wv, bv,
                  Wo, bo, q_batch, k_batch):
    """Host fallback (degenerate batch layouts)."""
    Q = (q_feat @ Wq + bq).reshape(N, H, D).transpose(1, 0, 2)
    K = (k_feat @ Wk + bk).reshape(N, H, D).transpose(1, 0, 2)
    V = (v_feat @ Wv + bv).reshape(N, H, D).transpose(1, 0, 2)
    scores = np.einsum("hnd,hmd->hnm", Q, K) / SCALE + pos_enc
    mask = q_batch[:, None] != k_batch[None, :]
    scores = np.where(mask[None], np.float32(-1e9), scores)
    scores = scores - scores.max(-1, keepdims=True)
    e = np.exp(scores)
    probs = e / e.sum(-1, keepdims=True)
    h = np.einsum("hnm,hmd->hnd", probs, V)
    h = h.transpose(1, 0, 2).reshape(N, OD)
    return (h @ Wo + bo).astype(np.float32)


def kernel(q_feat, k_feat, v_feat, pos_enc, Wq, bq, Wk, bk, Wv, bv, Wo, bo,
           q_batch, k_batch):
    global LAST_RESULTS
    args = dict(q_feat=np.asarray(q_feat, np.float32),
                k_feat=np.asarray(k_feat, np.float32),
                v_feat=np.asarray(v_feat, np.float32),
                pos_enc=np.asarray(pos_enc, np.float32),
                Wq=np.asarray(Wq, np.float32), bq=np.asarray(bq, np.float32),
                Wk=np.asarray(Wk, np.float32), bk=np.asarray(bk, np.float32),
                Wv=np.asarray(Wv, np.float32), bv=np.asarray(bv, np.float32),
                Wo=np.asarray(Wo, np.float32), bo=np.asarray(bo, np.float32),
                q_batch=np.asarray(q_batch), k_batch=np.asarray(k_batch))

    qbound, kbound = _bounds(args["q_batch"], args["k_batch"])
    nq_all = np.diff(qbound)
    nk_all = np.diff(kbound)
    if np.any((nq_all > 0) & (nk_all == 0)) or nq_all.max() == 0:
        # a batch with queries but no keys gets uniform attention over ALL
        # keys in the reference; fall back (never happens for real inputs)
        return _kernel_numpy(**args)

    NQ = int(nq_all.max())
    NKE = int(nk_all.max())
    NKP = max(128, ((NKE + 127) // 128) * 128)
    if NQ > 512 or NKP > 512:
        return _kernel_numpy(**args)

    has_bq = bool(np.any(args["bq"]))
    has_bk = bool(np.any(args["bk"]))
    has_bv = bool(np.any(args["bv"]))
    has_bo = bool(np.any(args["bo"]))

    nc = _build(NQ, NKP, NKE, has_bq, has_bk, has_bv, has_bo)

    NKC = NKP // 128
    NTD = OD // 128

    # ---- host-side sharding / layout / padding ----
    def pmaj(x):
        # [KT*128, n] -> [128, KT, n] partition-major (3-4KB DMA runs)
        kt = x.shape[0] // 128
        return np.ascontiguousarray(
            x.reshape(kt, 128, x.shape[1]).transpose(1, 0, 2))

    qfT = np.ascontiguousarray(args["q_feat"].T).astype(BF16_NP)
    kfT = np.ascontiguousarray(args["k_feat"].T).astype(BF16_NP)
    vfT = np.ascontiguousarray(args["v_feat"].T).astype(BF16_NP)
    wq8 = pmaj((args["Wq"] / SCALE).astype(BF16_NP))
    wkb = pmaj(args["Wk"].astype(BF16_NP))
    wvb = pmaj(args["Wv"].astype(BF16_NP))
    woh = pmaj(args["Wo"].astype(BF16_NP))

    in_maps = []
    for c in range(NCORES):
        qs, qe = int(qbound[c]), int(qbound[c + 1])
        ks, ke = int(kbound[c]), int(kbound[c + 1])
        nq, nk = qe - qs, ke - ks

        qfc = np.zeros((QD, NQ), BF16_NP)
        qfc[:, :nq] = qfT[:, qs:qe]
        kfc = np.zeros((QD, NKE), BF16_NP)
        kfc[:, :nk] = kfT[:, ks:ke]
        vfc = np.zeros((QD, NKP), BF16_NP)
        vfc[:, :nk] = vfT[:, ks:ke]
        qfc, kfc, vfc = pmaj(qfc), pmaj(kfc), pmaj(vfc)

        # posc holds exp(pos): 0 on masked/pad k rows, 1 on pad-q columns
        posc = np.zeros((H, NKP, NQ), BF16_NP)
        if nk > 0:
            posc[:, :nk, :] = 1.0
            posc[:, :nk, :nq] = np.exp(
                args["pos_enc"][:, qs:qe, ks:ke]).swapaxes(1, 2).astype(BF16_NP)
        # [H, NKP, NQ] -> [H, 128, NKC, NQ] p-major per head
        nkc = NKP // 128
        posc = np.ascontiguousarray(
            posc.reshape(H, nkc, 128, NQ).transpose(0, 2, 1, 3))

        m = {"qfT": qfc, "kfT": kfc, "vfT": vfc, "posc": posc,
             "wq": wq8, "wk": wkb, "wv": wvb, "woh": woh}
        if has_bq:
            m["bq"] = np.ascontiguousarray(
                (args["bq"] / SCALE).astype(np.float32).reshape(NTD, 128).T)
        if has_bk:
            m["bk"] = np.ascontiguousarray(
                args["bk"].astype(np.float32).reshape(NTD, 128).T)
        if has_bv:
            m["bv"] = args["bv"].astype(BF16_NP).reshape(1, OD)
        if has_bo:
            m["bo"] = np.ascontiguousarray(
                args["bo"].astype(np.float32).reshape(NTD, 128).T)
        in_maps.append(m)

    res = run_bass_kernel_spmd(nc, in_maps, core_ids=list(range(NCORES)),
                               trace=TRACE)
    LAST_RESULTS = res
    out = np.empty((N, OD), np.float32)
    for c in range(NCORES):
        qs, qe = int(qbound[c]), int(qbound[c + 1])
        if qe > qs:
            # out is [128, NTD, NQ] p-major -> [OD, NQ]
            o = res.results[c]["out"].transpose(1, 0, 2).reshape(OD, NQ)
            out[qs:qe, :] = o[:, :qe - qs].T.astype(np.float32)
    return out


# revision 76
# speedup vs baseline: 1.1589x; 1.0237x over previous
"""Trainium2 Bass kernel for nn_MultiHeadAttention_3796751090171 (sparse_attention).

Batch-parallel SPMD across 8 NeuronCores: q_batch/k_batch are SORTED, so the
cross-batch mask makes attention block-diagonal over batches, and there are
exactly B=8 batches for 8 cores. Core c computes batch c's queries against
batch c's keys for ALL 8 heads -- completely independent work, NO collectives.

Design (trace-driven, ~74us vs the 81.6us v1 baseline; the HAM power manager
caps sustained PE throughput at ~1.3GHz average, so wall time is dominated by
PE stream columns + the saturated ACT (exp) engine):
  - exact shapes: NQ = max batch q-count, K projection only computes the real
    key count (zeros memset once for the padded tail); PE matmul cost is
    output-columns x 1 cycle.
  - all bulk inputs host-packed partition-major ([128, KT, n]) so every DMA
    moves 2-4KB contiguous runs per partition (small-packet layouts measured
    only 50-90 GB/s/queue); loads split across the sync/gpsimd/scalar queues
    by need-time; posc streamed per-head round-robin over all 3 queues.
  - V projected directly in [k, d] layout (features stationary) -- no PE
    transposes; K/Q/V projections run contraction-major so each half-tensor
    DMA chunk is consumed as it lands (4 psum banks).
  - scores -> exp (ACT) -> *exp(pos) (DVE, host-precomputed exp(pos), 2x
    mode) -> AV accumulate [hT | Z] via a fused ones-column.
  - per head: 1/Z via the fast-reciprocal DVE uop straight off the AV psum
    (full-tile: the uop mishandles partition-base-64 row APs), broadcast
    across 64 partitions via a DRAM bounce with a stride-0 partition read,
    one DVE multiply normalizes into a head-PAIR tile; the output projection
    then runs K=128 per pair into 4 persistent psum banks (no serial tail).
  - software-pipelined slot schedule (LOOK_B/LOOK_C/D2LAG) keeps PE fed
    while the ACT exp stream and the per-head Z chain run LOOK slots behind.

PSUM budget (8 banks): 2 (K proj / score tiles) + 2 (Q,V proj / AV accum) +
4 (running paired output-projection accumulators).
"""

import functools
import math

import numpy as np
import ml_dtypes

import concourse.bass as bass
import concourse.tile as tile
from concourse import bacc, mybir
from concourse.bass_utils import run_bass_kernel_spmd

N = 3072
QD = 512
OD = 512
H = 8
D = 64
B = 8
NCORES = 8
SCALE = math.sqrt(D)

F32 = mybir.dt.float32
BF16 = mybir.dt.bfloat16
BF16_NP = ml_dtypes.bfloat16

TRACE = False
LAST_RESULTS = None

# schedule tuning knobs
import os as _os
WARM = int(_os.environ.get("KWARM", "0"))   # warmup matmuls while DMAs land
LOOK_B = int(_os.environ.get("KLB", "1"))   # slots: score pair -> exp+mult
LOOK_C = int(_os.environ.get("KLC", "4"))   # slots: score pair -> AV pair
D2LAG = int(_os.environ.get("KD2", "2"))    # slots: D1 -> D2 (zbc+normalize)


def _bounds(q_batch, k_batch):
    qb = np.asarray(q_batch).astype(np.int64)
    kb = np.asarray(k_batch).astype(np.int64)
    qbound = np.searchsorted(qb, np.arange(B + 1))
    kbound = np.searchsorted(kb, np.arange(B + 1))
    return qbound, kbound


@functools.lru_cache(maxsize=8)
def _build(NQ, NKP, NKE, has_bq, has_bk, has_bv, has_bo):
    nc = bacc.Bacc("TRN2", target_bir_lowering=False, debug=False,
                   num_devices=NCORES)

    KT = QD // 128     # 4 feature-contraction chunks
    NKC = NKP // 128   # k chunks
    NTD = OD // 128    # output-dim tiles for Q/K projections

    # All bulk inputs are host-packed p-major ([128, KT, n]: partition-
    # contiguous 3-4KB runs) so the DMA engines move full-size packets --
    # 832B-row layouts measured only 50-90 GB/s per queue, 3-4KB gets ~200+.
    qfT_d = nc.dram_tensor("qfT", [128, KT, NQ], BF16, kind="ExternalInput")
    kfT_d = nc.dram_tensor("kfT", [128, KT, NKE], BF16, kind="ExternalInput")
    vfT_d = nc.dram_tensor("vfT", [128, KT, NKP], BF16, kind="ExternalInput")
    posc_d = nc.dram_tensor("posc", [H, 128, NKC, NQ], BF16, kind="ExternalInput")
    wq_d = nc.dram_tensor("wq", [128, KT, OD], BF16, kind="ExternalInput")
    wk_d = nc.dram_tensor("wk", [128, KT, OD], BF16, kind="ExternalInput")
    wv_d = nc.dram_tensor("wv", [128, KT, OD], BF16, kind="ExternalInput")
    woh_d = nc.dram_tensor("woh", [128, KT, OD], BF16, kind="ExternalInput")
    bq_d = nc.dram_tensor("bq", [128, NTD], F32, kind="ExternalInput") if has_bq else None
    bk_d = nc.dram_tensor("bk", [128, NTD], F32, kind="ExternalInput") if has_bk else None
    bv_d = nc.dram_tensor("bv", [1, OD], BF16, kind="ExternalInput") if has_bv else None
    bo_d = nc.dram_tensor("bo", [128, NTD], F32, kind="ExternalInput") if has_bo else None
    out_d = nc.dram_tensor("out", [128, NTD, NQ], BF16, kind="ExternalOutput")
    import os
    DEBUG = bool(os.environ.get("KDBG"))
    if DEBUG:
        dbg_q = nc.dram_tensor("dbg_q", [128, NTD, NQ], BF16, kind="ExternalOutput")
        dbg_k = nc.dram_tensor("dbg_k", [128, NTD, NKP], BF16, kind="ExternalOutput")
        dbg_v = nc.dram_tensor("dbg_v", [128, NKC, H, D + 1], BF16, kind="ExternalOutput")
        dbg_ht = nc.dram_tensor("dbg_ht", [H, D + 1, NQ], F32, kind="ExternalOutput")
        dbg_em = nc.dram_tensor("dbg_em", [4, 128, 2 * NQ], BF16, kind="ExternalOutput")
        dbg_hn = nc.dram_tensor("dbg_hn", [H, D, NQ], BF16, kind="ExternalOutput")
        dbg_zr = nc.dram_tensor("dbg_zr", [H, NQ], F32, kind="ExternalOutput")
        dbg_zrb = nc.dram_tensor("dbg_zrb", [H, NQ], BF16, kind="ExternalOutput")

    with tile.TileContext(nc) as tc:
        with (
            tc.tile_pool(name="consts", bufs=1) as consts,
            tc.tile_pool(name="posp", bufs=4) as posp,
            tc.tile_pool(name="expp", bufs=4) as expp,
            tc.tile_pool(name="hp", bufs=3) as hp,
            tc.tile_pool(name="hnp", bufs=2) as hnp,
            tc.tile_pool(name="outp", bufs=4) as outp,
            tc.tile_pool(name="ps_s", bufs=2, space="PSUM") as ps_s,
            tc.tile_pool(name="ps_av", bufs=2, space="PSUM") as ps_av,
            tc.tile_pool(name="ps_o", bufs=1, space="PSUM") as ps_o,
            tc.tile_pool(name="dram", bufs=1, space="DRAM") as dramp,
        ):
            # ---------------- warmup: keep the PE busy from t=0 so the HAM
            # activity ramp (1.2 -> 2.4 GHz) starts before the projections.
            # row 64 of a [65, D] ones tile: partition base matches the Z row
            # of the AV accumulator for the K=1 broadcast matmul
            if WARM:
                wtile = consts.tile([128, 128], BF16, name="wtile")
                nc.vector.memset(wtile, 0.5)
                warm_ps = ps_s.tile([128, 512], F32, tag="pss", name="warm_ps")
                for wi in range(WARM):
                    nc.tensor.matmul(warm_ps[:, 0:128], wtile[:, :],
                                     wtile[:, :], start=(wi % 8 == 0),
                                     stop=(wi % 8 == 7 or wi == WARM - 1))
                warm_sb = consts.tile([1, 1], F32, name="warm_sb")
                nc.vector.tensor_copy(warm_sb[0:1, 0:1], warm_ps[0:1, 0:1])
                warm_d = dramp.tile([1, 1], F32, name="warm_d")
                nc.gpsimd.dma_start(out=warm_d[:, :], in_=warm_sb[0:1, 0:1])

            # ---------------- weights / features (DMA issue order matters:
            # K-proj inputs first, finely chunked, so the PE starts early)
            # two half-tensor tiles per input: 2KB p-major runs keep DMA
            # packets near full rate while halving the wait for the first
            # projection matmuls; queue order follows need-time
            def half_tiles(nm, n):
                return [consts.tile([128, 2, n], BF16, tag=f"{nm}{i}",
                                    name=f"{nm}{i}") for i in range(2)]

            wk_sb = half_tiles("wk", OD)
            kf_sb = half_tiles("kf", NKE)
            wq_sb = half_tiles("wq", OD)
            qf_sb = half_tiles("qf", NQ)
            wv_sb = half_tiles("wv", OD)
            vf_sb = half_tiles("vf", NKP)
            for eng, loads in (
                (nc.sync, [(wk_sb[0], wk_d, 0), (wk_sb[1], wk_d, 1),
                           (qf_sb[0], qfT_d, 0), (qf_sb[1], qfT_d, 1)]),
                (nc.scalar, [(kf_sb[0], kfT_d, 0), (kf_sb[1], kfT_d, 1),
                             (wq_sb[0], wq_d, 0), (wq_sb[1], wq_d, 1)]),
                (nc.gpsimd, [(wv_sb[0], wv_d, 0), (wv_sb[1], wv_d, 1),
                             (vf_sb[0], vfT_d, 0), (vf_sb[1], vfT_d, 1)]),
            ):
                for dst, srcd, i in loads:
                    eng.dma_start(out=dst, in_=srcd[:, 2 * i:2 * i + 2, :])

            # Wo in natural [in, out] layout: rows 128*hp..128*hp+127 hold the
            # head-PAIR hp's input dims, enabling K=128 paired oproj matmuls
            wo_sb = consts.tile([128, NTD, OD], BF16, name="wo_sb")
            nc.gpsimd.dma_start(out=wo_sb, in_=woh_d[:, :, :])

            bias_sb = {}
            for nm, dd in (("bq", bq_d), ("bk", bk_d), ("bo", bo_d)):
                if dd is not None:
                    t = consts.tile([128, NTD], F32, tag=f"b_{nm}", name=f"b_{nm}")
                    nc.gpsimd.dma_start(out=t, in_=dd[:, :])
                    bias_sb[nm] = t
            if bv_d is not None:
                bv_sb = consts.tile([1, OD], BF16, name="bv_sb")
                nc.gpsimd.dma_start(out=bv_sb, in_=bv_d[:, :])
                ones1 = consts.tile([1, 128], BF16, name="ones1")
                nc.vector.memset(ones1, 1.0)

            # projected tensors
            KT_f = consts.tile([128, NTD, NKP], BF16, name="KT_f")
            QT_f = consts.tile([128, NTD, NQ], BF16, name="QT_f")
            V_sb = consts.tile([128, NKC, H, D + 1], BF16, name="V_sb")
            # ones column for the fused-Z row of the AV matmul
            nc.vector.memset(V_sb[:, :, :, D], 1.0)

            # ---------------- K / Q projections (out = W^T X, d on partitions)
            def proj_tmajor(f_sb, w_sb, dst, ncols, bias):
                # contraction-major: each DMA chunk t is consumed as soon as
                # it lands; needs all 4 psum banks (2 from each pool)
                tiles = [ps_s.tile([128, 512], F32, tag="pss", name="pp0"),
                         ps_s.tile([128, 512], F32, tag="pss", name="pp1"),
                         ps_av.tile([128, 512], F32, tag="avt", name="pp2"),
                         ps_av.tile([128, 512], F32, tag="avt", name="pp3")]
                for t in range(KT):
                    for td in range(NTD):
                        dsl = slice(128 * td, 128 * (td + 1))
                        nc.tensor.matmul(tiles[td][:, 0:ncols],
                                         w_sb[t // 2][:, t % 2, dsl],
                                         f_sb[t // 2][:, t % 2, 0:ncols],
                                         start=(t == 0), stop=(t == KT - 1))
                for td in range(NTD):
                    ps = tiles[td]
                    if bias is not None:
                        nc.scalar.activation(dst[:, td, 0:ncols], ps[:, 0:ncols],
                                             mybir.ActivationFunctionType.Identity,
                                             bias=bias[:, td:td + 1])
                    elif td % 2 == 0:
                        nc.scalar.copy(dst[:, td, 0:ncols], ps[:, 0:ncols])
                    else:
                        nc.vector.tensor_copy(dst[:, td, 0:ncols], ps[:, 0:ncols])

            if NKE < NKP:
                # zero the k-positions beyond the real key count once; the
                # K projection then only computes NKE columns
                nc.gpsimd.memset(KT_f[:, :, NKE:NKP], 0.0)
            proj_tmajor(kf_sb, wk_sb, KT_f, NKE, bias_sb.get("bk"))
            proj_tmajor(qf_sb, wq_sb, QT_f, NQ, bias_sb.get("bq"))

            # ---------------- V projection, direct [k, d] layout, t-major:
            # out[k, d] += vf_chunk^T @ wv_chunk  (features stationary)
            vtiles = [ps_s.tile([128, 512], F32, tag="pss", name="vp0"),
                      ps_s.tile([128, 512], F32, tag="pss", name="vp1"),
                      ps_av.tile([128, 512], F32, tag="avt", name="vp2"),
                      ps_av.tile([128, 512], F32, tag="avt", name="vp3")]
            for t in range(KT):
                for c in range(NKC):
                    ksl = slice(128 * c, 128 * (c + 1))
                    nc.tensor.matmul(vtiles[c][:, 0:OD],
                                     vf_sb[t // 2][:, t % 2, ksl],
                                     wv_sb[t // 2][:, t % 2, :],
                                     start=(t == 0),
                                     stop=(t == KT - 1 and bv_d is None))
            for c in range(NKC):
                if bv_d is not None:
                    nc.tensor.matmul(vtiles[c][:, 0:OD], ones1[:, 0:128],
                                     bv_sb[:, :], start=False, stop=True)
                src = vtiles[c][:, 0:OD].rearrange("p (h d) -> p h d", h=H)
                if c % 2 == 0:
                    nc.scalar.copy(V_sb[:, c, :, 0:D], src)
                else:
                    nc.vector.tensor_copy(V_sb[:, c, :, 0:D], src)

            # ---------------- attention: slots = (head, kchunk-pair) --------
            NP = NKC // 2          # kchunk pairs per head (2)
            S = H * NP             # 16 slots
            st_ps = {}
            expm = {}
            avps = {}
            hts = {}
            pos_tiles = {}
            pos_eng = [nc.sync, nc.gpsimd, nc.gpsimd]

            def issue_pos(h):
                t = posp.tile([128, NKC, NQ], BF16, tag="pos", name="pos")
                pos_eng[h % 3].dma_start(out=t, in_=posc_d[h])
                pos_tiles[h] = t

            for h in range(min(3, H)):
                issue_pos(h)

            def stageA(s):  # score pair matmuls (two f32 psum tiles)
                h, p = s // NP, s % NP
                if p == 0 and h + 3 < H:
                    issue_pos(h + 3)
                po = D * (h % 2)
                tiles = []
                for ci in (0, 1):
                    c = 2 * p + ci
                    ksl = slice(128 * c, 128 * (c + 1))
                    pst = ps_s.tile([128, NQ], F32, tag="pss", name="pst")
                    nc.tensor.matmul(pst[:, 0:NQ],
                                     KT_f[po:po + D, h // 2, ksl],
                                     QT_f[po:po + D, h // 2, 0:NQ],
                                     start=True, stop=True)
                    tiles.append(pst)
                st_ps[s] = tiles

            def stageD1(h):  # 1/Z off the AV psum; bounce it via DRAM to
                # broadcast across 64 partitions (stride-0 partition read) --
                # no PE matmul, no ACT convert.
                # full-tile recip: the custom DVE uop mishandles partition-
                # base-64 single-row APs; lanes are parallel so [65,NQ] costs
                # the same and row 64 gives 1/Z (rows 0..63 are junk, unread)
                zr = hp.tile([D + 1, NQ], F32, tag="zr", name="zr")
                nc.vector.reciprocal_approx_fast(zr[:, :], avps[h][:, 0:NQ])
                zr_d = dramp.tile([1, NQ], F32, tag=f"zrd{h}", name=f"zrd{h}")
                nc.gpsimd.dma_start(out=zr_d[0:1, :], in_=zr[D:D + 1, :])
                zr_ap = zr_d[:, :]
                zbc = hp.tile([D, NQ], F32, tag="zbc", name="zbc")
                nc.sync.dma_start(
                    out=zbc[:, :],
                    in_=bass.AP(tensor=zr_ap.tensor, offset=zr_ap.offset,
                                ap=[[0, D], [1, NQ]]))
                ht = hp.tile([D + 1, NQ], F32, tag="hT", name="ht")
                if h % 4 == 0:
                    nc.scalar.copy(ht[:, :], avps[h][:, 0:NQ])
                else:
                    nc.vector.tensor_copy(ht[:, :], avps[h][:, 0:NQ])
                del avps[h]
                if DEBUG:
                    nc.sync.dma_start(out=dbg_ht.ap()[h], in_=ht[:, :])
                    nc.sync.dma_start(out=dbg_zr.ap()[h:h + 1, :],
                                      in_=zr[D:D + 1, :])
                hts[h] = (ht, zbc)

            def stageB(s):  # exp x2 (ACT) + paired posc multiply (DVE)
                h, p = s // NP, s % NP
                ex = expp.tile([128, 2 * NQ], BF16, tag="expr", name="ex")
                for ci in (0, 1):
                    nc.scalar.activation(ex[:, NQ * ci:NQ * (ci + 1)],
                                         st_ps[s][ci][:, 0:NQ],
                                         mybir.ActivationFunctionType.Exp)
                del st_ps[s]
                em = expp.tile([128, 2 * NQ], BF16, tag="expm", name="em")
                nc.vector.tensor_mul(
                    em[:, :].rearrange("p (c q) -> p c q", c=2),
                    ex[:, :].rearrange("p (c q) -> p c q", c=2),
                    pos_tiles[h][:, 2 * p:2 * p + 2, :])
                if DEBUG and s < 4:
                    nc.sync.dma_start(out=dbg_em.ap()[s], in_=em[:, :])
                expm[s] = em

            def stageC(s):  # AV pair (accumulate [hT | Z] per head)
                h, p = s // NP, s % NP
                if p == 0:
                    avps[h] = ps_av.tile([D + 1, NQ], F32, tag="avt", name="avt")
                for ci in (0, 1):
                    c = 2 * p + ci
                    nc.tensor.matmul(avps[h][:, 0:NQ], V_sb[:, c, h, :],
                                     expm[s][:, NQ * ci:NQ * (ci + 1)],
                                     start=(c == 0), stop=(c == NKC - 1))
                del expm[s]

            hn2 = {}

            def stageD2(h):  # normalize into the pair tile
                ht, zbc = hts.pop(h)
                hp_ = h // 2
                if h % 2 == 0:
                    hn2[hp_] = hnp.tile([128, NQ], BF16, tag="hTn", name="hn")
                po = D * (h % 2)
                nc.vector.tensor_mul(hn2[hp_][po:po + D, :], ht[0:D, :],
                                     zbc[:, :])
                if DEBUG:
                    nc.sync.dma_start(out=dbg_hn.ap()[h],
                                      in_=hn2[hp_][po:po + D, :])

            def stageD3(hp_):  # paired output projection (K=128)
                hn = hn2.pop(hp_)
                last = hp_ == H // 2 - 1
                for oc in range(NTD):
                    nc.tensor.matmul(ps_o_t[oc][:, 0:NQ],
                                     wo_sb[:, hp_, 128 * oc:128 * (oc + 1)],
                                     hn[:, :],
                                     start=(hp_ == 0), stop=(hp_ == H // 2 - 1),
                                     skip_group_check=True)
                    if last:
                        store_out(oc)

            ps_o_t = [ps_o.tile([128, NQ], F32, tag=f"o{oc}", name=f"po{oc}")
                      for oc in range(NTD)]

            def store_out(oc):
                osb = outp.tile([128, NQ], BF16, tag="osb", name="osb")
                if bo_d is not None:
                    nc.scalar.activation(osb[:, :], ps_o_t[oc][:, 0:NQ],
                                         mybir.ActivationFunctionType.Identity,
                                         bias=bias_sb["bo"][:, oc:oc + 1])
                elif oc % 2 == 0:
                    nc.scalar.copy(osb[:, :], ps_o_t[oc][:, 0:NQ])
                else:
                    nc.vector.tensor_copy(osb[:, :], ps_o_t[oc][:, 0:NQ])
                (nc.sync if oc % 2 == 0 else nc.gpsimd).dma_start(
                    out=out_d[:, oc, 0:NQ], in_=osb[:, :])

            # D1(h) fires in the same slot as its p1 AV pair (right after it);
            # D2(h) D2LAG slots later (PE visits the zbc matmul after slots of
            # score/AV work, hiding the recip->zrb latency); D3 (paired oproj)
            # one slot after the odd head's D2.
            d2base = 2 * NP + 1 + LOOK_C + D2LAG  # slot of D2(h=NP-...)? h=1
            # D1(h) at 2h+1+LOOK_C; D2(h) at 2h+1+LOOK_C+D2LAG;
            # D3(hp) at 2(2hp+1)+1+LOOK_C+D2LAG+1
            for s in range(S + LOOK_C + D2LAG + 6):
                if s < S:
                    stageA(s)
                if 0 <= s - LOOK_B < S:
                    stageB(s - LOOK_B)
                sd = s - LOOK_C  # slot whose AV-pair is issued now
                if 0 <= sd < S:
                    stageC(sd)
                    if sd % NP == NP - 1:
                        stageD1(sd // NP)
                o2 = s - (1 + LOOK_C + D2LAG)
                if o2 >= 0 and o2 % 2 == 0 and o2 // 2 < H:
                    stageD2(o2 // 2)
                o3 = s - (3 + LOOK_C + D2LAG + 1)
                if o3 >= 0 and o3 % 4 == 0 and o3 // 4 < H // 2:
                    stageD3(o3 // 4)

            if DEBUG:
                nc.sync.dma_start(out=dbg_q.ap(), in_=QT_f[:, :, :])
                nc.sync.dma_start(out=dbg_k.ap(), in_=KT_f[:, :, :])
                nc.sync.dma_start(out=dbg_v.ap(), in_=V_sb[:, :, :, :])

    nc.compile()
    return nc


def _kernel_numpy(q_feat, k_feat, v_feat, pos_enc, Wq, bq, Wk, bk, Wv, bv,
                  Wo, bo, q_batch, k_batch):
    """Host fallback (degenerate batch layouts)."""
    Q = (q_feat @ Wq + bq).reshape(N, H, D).transpose(1, 0, 2)
    K = (k_feat @ Wk + bk).reshape(N, H, D).transpose(1, 0, 2)
    V = (v_feat @ Wv + bv).reshape(N, H, D).transpose(1, 0, 2)
    scores = np.einsum("hnd,hmd->hnm", Q, K) / SCALE + pos_enc
    mask = q_batch[:, None] != k_batch[None, :]
    scores = np.where(mask[None], np.float32(-1e9), scores)
    scores = scores - scores.max(-1, keepdims=True)
    e = np.exp(scores)
    probs = e / e.sum(-1, keepdims=True)
    h = np.einsum("hnm,hmd->hnd", probs, V)
    h = h.transpose(1, 0, 2).reshape(N, OD)
    return (h @ Wo + bo).astype(np.float32)


def kernel(q_feat, k_feat, v_feat, pos_enc, Wq, bq, Wk, bk, Wv, bv, Wo, bo,
           q_batch, k_batch):
    global LAST_RESULTS
    args = dict(q_feat=np.asarray(q_feat, np.float32),
                k_feat=np.asarray(k_feat, np.float32),
                v_feat=np.asarray(v_feat, np.float32),
                pos_enc=np.asarray(pos_enc, np.float32),
                Wq=np.asarray(Wq, np.float32), bq=np.asarray(bq, np.float32),
                Wk=np.asarray(Wk, np.float32), bk=np.asarray(bk, np.float32),
                Wv=np.asarray(Wv, np.float32), bv=np.asarray(bv, np.float32),
                Wo=np.asarray(Wo, np.float32), bo=np.asarray(bo, np.float32),
                q_batch=np.asarray(q_batch), k_batch=np.asarray(k_batch))

    qbound, kbound = _bounds(args["q_batch"], args["k_batch"])
    nq_all = np.diff(qbound)
    nk_all = np.diff(kbound)
    if np.any((nq_all > 0) & (nk_all == 0)) or nq_all.max() == 0:
        # a batch with queries but no keys gets uniform attention over ALL
        # keys in the reference; fall back (never happens for real inputs)
        return _kernel_numpy(**args)

    NQ = int(nq_all.max())
    NKE = int(nk_all.max())
    NKP = max(128, ((NKE + 127) // 128) * 128)
    if NQ > 512 or NKP > 512:
        return _kernel_numpy(**args)

    has_bq = bool(np.any(args["bq"]))
    has_bk = bool(np.any(args["bk"]))
    has_bv = bool(np.any(args["bv"]))
    has_bo = bool(np.any(args["bo"]))

    nc = _build(NQ, NKP, NKE, has_bq, has_bk, has_bv, has_bo)

    NKC = NKP // 128
    NTD = OD // 128

    # ---- host-side sharding / layout / padding ----
    def pmaj(x):
        # [KT*128, n] -> [128, KT, n] partition-major (3-4KB DMA runs)
        kt = x.shape[0] // 128
        return np.ascontiguousarray(
            x.reshape(kt, 128, x.shape[1]).transpose(1, 0, 2))

    qfT = np.ascontiguousarray(args["q_feat"].T).astype(BF16_NP)
    kfT = np.ascontiguousarray(args["k_feat"].T).astype(BF16_NP)
    vfT = np.ascontiguousarray(args["v_feat"].T).astype(BF16_NP)
    wq8 = pmaj((args["Wq"] / SCALE).astype(BF16_NP))
    wkb = pmaj(args["Wk"].astype(BF16_NP))
    wvb = pmaj(args["Wv"].astype(BF16_NP))
    woh = pmaj(args["Wo"].astype(BF16_NP))

    in_maps = []
    for c in range(NCORES):
        qs, qe = int(qbound[c]), int(qbound[c + 1])
        ks, ke = int(kbound[c]), int(kbound[c + 1])
        nq, nk = qe - qs, ke - ks

        qfc = np.zeros((QD, NQ), BF16_NP)
        qfc[:, :nq] = qfT[:, qs:qe]
        kfc = np.zeros((QD, NKE), BF16_NP)
        kfc[:, :nk] = kfT[:, ks:ke]
        vfc = np.zeros((QD, NKP), BF16_NP)
        vfc[:, :nk] = vfT[:, ks:ke]
        qfc, kfc, vfc = pmaj(qfc), pmaj(kfc), pmaj(vfc)

        # posc holds exp(pos): 0 on masked/pad k rows, 1 on pad-q columns
        posc = np.zeros((H, NKP, NQ), BF16_NP)
        if nk > 0:
            posc[:, :nk, :] = 1.0
            posc[:, :nk, :nq] = np.exp(
                args["pos_enc"][:, qs:qe, ks:ke]).swapaxes(1, 2).astype(BF16_NP)
        # [H, NKP, NQ] -> [H, 128, NKC, NQ] p-major per head
        nkc = NKP // 128
        posc = np.ascontiguousarray(
            posc.reshape(H, nkc, 128, NQ).transpose(0, 2, 1, 3))

        m = {"qfT": qfc, "kfT": kfc, "vfT": vfc, "posc": posc,
             "wq": wq8, "wk": wkb, "wv": wvb, "woh": woh}
        if has_bq:
            m["bq"] = np.ascontiguousarray(
                (args["bq"] / SCALE).astype(np.float32).reshape(NTD, 128).T)
        if has_bk:
            m["bk"] = np.ascontiguousarray(
                args["bk"].astype(np.float32).reshape(NTD, 128).T)
        if has_bv:
            m["bv"] = args["bv"].astype(BF16_NP).reshape(1, OD)
        if has_bo:
            m["bo"] = np.ascontiguousarray(
                args["bo"].astype(np.float32).reshape(NTD, 128).T)
        in_maps.append(m)

    res = run_bass_kernel_spmd(nc, in_maps, core_ids=list(range(NCORES)),
                               trace=TRACE)
    LAST_RESULTS = res
    out = np.empty((N, OD), np.float32)
    for c in range(NCORES):
        qs, qe = int(qbound[c]), int(qbound[c + 1])
        if qe > qs:
            # out is [128, NTD, NQ] p-major -> [OD, NQ]
            o = res.results[c]["out"].transpose(1, 0, 2).reshape(OD, NQ)
            out[qs:qe, :] = o[:, :qe - qs].T.astype(np.float32)
    return out
